# revision 26
# baseline (speedup 1.0000x reference)
"""Trainium2 Bass kernel for nn_CombineNode_7395933684091 (gnn_message_passing).

Hierarchy: 128 leaf terms (each D=1024 -> H=32), 16 internal terms
(concat of 8 children hiddens, 256 -> 32), 1 root (concat of 16
internal hiddens, 512 -> 32); every term also has a 1-dim predict head.
All matmuls followed by tanh.

Strategy: data-parallel over batch across 8 cores (Bc = 1024 rows per
core), weights replicated. On-chip layout keeps hidden features on the
PARTITION axis ("h^T layout": tiles are [features, batch]), so every
level's contraction is a natural PE matmul and the child-concat is just
stacking partition tiles.

All matmul operands are bf16 and every stationary operand is padded to
128 columns: FWL (fast weight load) then applies to every LDWEIGHTS,
and the PE never pays the ~95ns FWL<->non-FWL mode-switch penalty
(measured on the 64-col internal-level matmuls). PSUM stays fp32;
biases/activations/outputs stay fp32.

Inputs are host-packed into exact SBUF layouts so every load is a few
large contiguous DMAs split across both HWDGE queues (SP: weight
panels, ACT: x + combined consts) - each extra dma_start costs
~0.6-1.2us of queue issue time, so constants ride in two combined
tensors.

Leaf level: 4 panels x 8 groups (4 leaves) x 8 k-chunk accumulated
[128,128]x[128,512] matmuls. The per-term predict heads ride along as
extra block-diagonal columns fused into the internal-level stationary
operand (cw) and the root-level stationary operand (rw2), so they cost
no extra PE streaming. Internal-level (comb) matmuls trail their tanh
inputs by 2.5+ leaf groups (deferred emission), so the PE never waits
on the scalar engine; the last panel's root chain for batch-half 0 is
deferred into half 1's leaf stream for the same reason. Leaf
predictions land column-packed in an [8, 2*16*512] tile (engine
partition windows must be 32-aligned) and are unpacked on host.
"""

import numpy as np

B, D, H = 8192, 1024, 32
L, I, CPI = 128, 16, 8
NCORES = 8
BC = B // NCORES      # 1024 batch rows per core
BN = 512              # batch tile width (one PSUM bank of f32)
NBH = BC // BN        # 2 batch halves
KC = D // 128         # 8 contraction chunks for the leaf level
NPANEL = 4            # leaf panels (8 groups of 4 leaves each)
GPP = 8               # groups per panel
NOUT = L + I + 1      # 145
CWN = NPANEL * 2 * KC * 128   # 4096: cw region cols (32 chunks x 128)
CWALL = CWN + 512 + 128       # + rw2 (4x128) + rootwp block

_CACHE = {}


def _build_nc():
    from contextlib import ExitStack

    import concourse.mybir as mybir
    import concourse.tile as tile
    from concourse import bacc

    f32 = mybir.dt.float32
    bf16 = mybir.dt.bfloat16
    Tanh = mybir.ActivationFunctionType.Tanh

    nc = bacc.Bacc("TRN2", target_bir_lowering=False, debug=False)

    # xt: [128, bn*4096 + k*512 + c] = x[bn*512+c, k*128+p] (per-core slice)
    xt = nc.dram_tensor("xt", [128, NBH * KC * BN], bf16, kind="ExternalInput")
    # lw: [128, panel*8192 + k*1024 + g*128 + j]
    lw = nc.dram_tensor("lw", [128, NPANEL * KC * 1024], bf16, kind="ExternalInput")
    # cwall = cw (32 chunks x 128) | rw2 (4 x 128) | rootwp block (128)
    cwall = nc.dram_tensor("cwall", [128, CWALL], bf16, kind="ExternalInput")
    # biasall = lb (32) | intb (4) | lbp8 (16) | intbp | rootb | rootbp
    biasall = nc.dram_tensor("biasall", [128, 55], f32, kind="ExternalInput")
    # leaf predicts, column-packed: outl[r, (bn*16 + i)*512 + c] is leaf
    # 8i+r at batch bn*512+c
    outl = nc.dram_tensor("outl", [8, NBH * I * BN], f32, kind="ExternalOutput")
    # int predicts rows 0:16, root predict row 16
    outi = nc.dram_tensor("outi", [17, BC], f32, kind="ExternalOutput")

    mm = nc.tensor.matmul

    with tile.TileContext(nc) as tc, ExitStack() as ctx:
        consts = ctx.enter_context(tc.tile_pool(name="consts", bufs=1))
        wpool = ctx.enter_context(tc.tile_pool(name="wpool", bufs=3))
        work = ctx.enter_context(tc.tile_pool(name="work", bufs=5))
        keep = ctx.enter_context(tc.tile_pool(name="keep", bufs=1))
        psum = ctx.enter_context(tc.tile_pool(name="psum", bufs=1, space="PSUM"))

        # --- PE pre-warm: ~2.8us of dummy matmuls unthrottles the HAM clock
        # gate (PE boots at 1.2 GHz; 3.4us of sustained activity -> 2.4 GHz).
        # A gpsimd memset (done during the engine preamble) feeds the tile so
        # no DMA gates the first matmul.
        warm_c = consts.tile([128, 128], bf16, name="warm_c")
        nc.gpsimd.memset(warm_c[:], 0.0)
        pwarm = psum.tile([128, 128], f32, tag="pcomb", bufs=3, name="pwarm")
        for _ in range(26):
            mm(pwarm[:], warm_c[:], warm_c[:], start=True, stop=True,
               skip_group_check=True)

        # --- loads. ACT queue: x chunks + consts, ordered by first use;
        # SP queue: weight panels. bn0's x and panel 0's weights stream in
        # k-chunks so the first wave matmul only waits ~one chunk.
        xt_sb = consts.tile([128, NBH * KC * BN], bf16, name="xt_sb")
        for k in range(3):
            nc.scalar.dma_start(xt_sb[:, k * 512:(k + 1) * 512],
                                xt[:, k * 512:(k + 1) * 512])
        ball_sb = consts.tile([128, 55], f32, name="ball_sb")
        nc.scalar.dma_start(ball_sb[:], biasall[:])
        nc.scalar.dma_start(xt_sb[:, 1536:2048], xt[:, 1536:2048])
        cwall_sb = consts.tile([128, CWALL], bf16, name="cwall_sb")
        nc.scalar.dma_start(cwall_sb[:, 0:1024], cwall[:, 0:1024])
        for k in range(4, KC):
            nc.scalar.dma_start(xt_sb[:, k * 512:(k + 1) * 512],
                                xt[:, k * 512:(k + 1) * 512])
        nc.scalar.dma_start(xt_sb[:, 4096:8192], xt[:, 4096:8192])
        nc.scalar.dma_start(cwall_sb[:, 1024:CWALL], cwall[:, 1024:CWALL])
        # panels 1-2 also ride the ACT queue, POSITIONED AFTER bn1's x and
        # the cw table: the serial HWDGE ring is the only way to keep their
        # 4MB from stealing HBM bandwidth from the startup-critical loads
        # (they aren't needed until ~35us/~55us)

        lb_sb = ball_sb[:, 0:32]
        intb_sb = ball_sb[:, 32:36]
        lbp8_sb = ball_sb[0:8, 36:52]
        intbp_sb = ball_sb[0:16, 52:53]
        rootb_sb = ball_sb[0:32, 53:54]
        rootbp_sb = ball_sb[0:1, 54:55]

        wps = {}
        for p in (1, 2):
            wps[p] = wpool.tile([128, KC * 1024], bf16, tag="wpanel", name=f"wp{p}")
            nc.scalar.dma_start(wps[p][:], lw[:, p * 8192:(p + 1) * 8192])

        # SP queue: only panel 0, front-loaded in k-chunks (full HBM share
        # against the ACT queue's x stream); panel 3 in the loop (its
        # buffer frees after panel 0).
        wps[0] = wpool.tile([128, KC * 1024], bf16, tag="wpanel", name="wp0")
        nc.sync.dma_start(wps[0][:, 0:1024], lw[:, 0:1024])
        nc.sync.dma_start(wps[0][:, 1024:2048], lw[:, 1024:2048])
        nc.sync.dma_start(wps[0][:, 2048:4096], lw[:, 2048:4096])
        nc.sync.dma_start(wps[0][:, 4096:8192], lw[:, 4096:8192])

        # leaf predictions, column-packed (bn-major) to keep every
        # activation write at partition base 0
        lpp = keep.tile([8, NBH * I * BN], f32, name="lpp")
        intp_sb = keep.tile([16, BC], f32, name="intp_sb")
        rootp_sb = keep.tile([1, BC], f32, name="rootp_sb")

        inth = {}  # (panel, bn) -> [128, BN] tile: int nodes 4p..4p+3 h^T

        def leaf_mm(wp, bn, gl, k, pg):
            mm(
                pg[:],
                wp[:, k * 1024 + gl * 128:k * 1024 + (gl + 1) * 128],
                xt_sb[:, bn * 4096 + k * BN:bn * 4096 + (k + 1) * BN],
                start=(k == 0),
                stop=(k == KC - 1),
            )

        def leaf_tanh(p, bn, gl, pg):
            lh = work.tile([128, BN], bf16, tag="lh", name=f"lh{p}{bn}{gl}")
            nc.scalar.activation(
                lh[:], pg[:], Tanh, bias=lb_sb[:, GPP * p + gl:GPP * p + gl + 1]
            )
            return lh

        def comb_mms(p, bn, il, lh0, lh1):
            """Fused internal-trans + leaf-predict matmul pair.

            pcomb rows 0:32 accumulate node (4p+il)'s hidden pre-activation
            over its two child groups; rows 32:40 pick up the 8 leaf predict
            dots via the block-diagonal columns (rest of the 128-col padded
            stationary is zero)."""
            i = 4 * p + il
            pcomb = psum.tile([128, BN], f32, tag="pcomb", bufs=3,
                              name=f"pc{p}{bn}{il}")
            for j, lh in ((0, lh0), (1, lh1)):
                mm(
                    pcomb[:],
                    cwall_sb[:, (2 * i + j) * 128:(2 * i + j + 1) * 128],
                    lh[:],
                    start=(j == 0),
                    stop=(j == 1),
                    skip_group_check=True,
                )
            return pcomb

        def comb_ith(p, bn, il, ith, pcomb):
            nc.scalar.activation(
                ith[32 * il:32 * il + 32, :],
                pcomb[0:32, :],
                Tanh,
                bias=intb_sb[32 * il:32 * il + 32, p:p + 1],
            )

        def comb_lpp(p, bn, il, pcomb):
            i = 4 * p + il
            nc.scalar.activation(
                lpp[0:8, (bn * I + i) * BN:(bn * I + i + 1) * BN],
                pcomb[32:40, :],
                Tanh,
                bias=lbp8_sb[:, i:i + 1],
            )

        def emit_comb(p, bn, il, ith, lh0, lh1):
            pcomb = comb_mms(p, bn, il, lh0, lh1)
            comb_ith(p, bn, il, ith, pcomb)
            comb_lpp(p, bn, il, pcomb)

        def flush_outl(p, bn, eng):
            lo = (bn * I + 4 * p) * BN
            hi = (bn * I + 4 * (p + 1)) * BN
            eng.dma_start(outl[0:8, lo:hi], lpp[0:8, lo:hi])

        def make_deferred(p, bn, ith, lh4, lh5, lh6, lh7):
            # il2 + il3 combs + the predict flush for (p, bn), emitted while
            # the NEXT half's leaf stream occupies the PE (their tanh inputs
            # are then guaranteed ready)
            def deferred():
                emit_comb(p, bn, 2, ith, lh4, lh5)
                emit_comb(p, bn, 3, ith, lh6, lh7)
                flush_outl(p, bn, nc.gpsimd)
            return deferred

        def root_part_a(bn, ith, lh4, lh5, lh6, lh7):
            """Last panel's il2/il3 combs + root accumulation q=0..2 + the
            ith activations. Root q0..q2 bridge the tanh-g7 wait."""
            p = NPANEL - 1
            pc2 = comb_mms(p, bn, 2, lh4, lh5)
            prc = psum.tile([128, BN], f32, tag="pg", bufs=5, name=f"prc{bn}")
            for q in range(NPANEL - 1):
                mm(prc[:], cwall_sb[:, CWN + 128 * q:CWN + 128 * (q + 1)],
                   inth[(q, bn)][:], start=(q == 0), stop=False,
                   skip_group_check=True)
            pc3 = comb_mms(p, bn, 3, lh6, lh7)
            comb_ith(p, bn, 2, ith, pc2)
            comb_ith(p, bn, 3, ith, pc3)
            return prc, pc2, pc3

        def root_part_b(bn, ith, prc, pc2, pc3):
            """Root q3 + the predict/root activation chain + output DMAs.
            Flushed two groups after part A so q3/prp never wait on ACT."""
            eng = nc.scalar if bn == NBH - 1 else nc.gpsimd
            p = NPANEL - 1
            mm(prc[:], cwall_sb[:, CWN + 384:CWN + 512], ith[:], start=False,
               stop=True, skip_group_check=True)
            rh = work.tile([32, BN], bf16, tag="rh", name=f"rh{bn}")
            nc.scalar.activation(rh[:], prc[0:32, :], Tanh,
                                 bias=rootb_sb[:, 0:1])
            prp = psum.tile([128, BN], f32, tag="pg", bufs=5, name=f"prp{bn}")
            mm(prp[:], cwall_sb[0:32, CWN + 512:CWN + 640], rh[:],
               start=True, stop=True, skip_group_check=True)
            comb_lpp(p, bn, 2, pc2)
            comb_lpp(p, bn, 3, pc3)
            flush_outl(p, bn, eng)
            nc.scalar.activation(
                intp_sb[:, bn * BN:bn * BN + BN], prc[32:48, :],
                Tanh, bias=intbp_sb[:, 0:1],
            )
            eng.dma_start(
                outi[0:16, bn * BN:bn * BN + BN],
                intp_sb[:, bn * BN:bn * BN + BN],
            )
            nc.scalar.activation(
                rootp_sb[0:1, bn * BN:bn * BN + BN], prp[0:1, :], Tanh,
                bias=rootbp_sb[:, 0:1],
            )
            eng.dma_start(
                outi[16:17, bn * BN:bn * BN + BN],
                rootp_sb[0:1, bn * BN:bn * BN + BN],
            )

        def root_section(bn, ith, lh4, lh5, lh6, lh7):
            prc, pc2, pc3 = root_part_a(bn, ith, lh4, lh5, lh6, lh7)
            root_part_b(bn, ith, prc, pc2, pc3)

        pending = []   # deferred combs/flush from the previous (p, bn), at g1
        pending3 = []  # deferred root part B (bn0), flushed at g3

        # --- leaf + internal levels ----------------------------------------
        for p in range(NPANEL):
            if p in wps:
                wp = wps[p]
            else:
                wp = wpool.tile([128, KC * 1024], bf16, tag="wpanel", name=f"wp{p}")
                nc.sync.dma_start(wp[:], lw[:, p * 8192:(p + 1) * 8192])
            for bn in range(NBH):
                ith = keep.tile([128, BN], bf16, tag=f"inth{p}{bn}",
                                name=f"inth{p}{bn}")
                lhs = {}

                if p == 0 and bn == 0:
                    # k-outer waves (5 then 3 groups): matmuls start as soon
                    # as the first xt/wp chunks land, and each arriving chunk
                    # feeds a full wave of matmuls. il0/il1 combs run between
                    # the waves; il2/il3 are deferred like everywhere else.
                    pgs = {}
                    for g0, cnt in ((0, 5), (5, 3)):
                        for q in range(cnt):
                            pgs[g0 + q] = psum.tile(
                                [128, BN], f32, tag="pg", bufs=5,
                                name=f"pgko{g0 + q}")
                        for k in range(KC):
                            for q in range(cnt):
                                leaf_mm(wp, bn, g0 + q, k, pgs[g0 + q])
                        if g0 == 0:
                            for g in (0, 1, 2, 3):
                                lhs[g] = leaf_tanh(p, bn, g, pgs[g])
                            for il in (0, 1):
                                emit_comb(p, bn, il, ith,
                                          lhs[2 * il], lhs[2 * il + 1])
                            lhs[4] = leaf_tanh(p, bn, 4, pgs[4])
                        else:
                            for g in (5, 6, 7):
                                lhs[g] = leaf_tanh(p, bn, g, pgs[g])
                else:
                    # in-loop combs trail their tanh inputs by 2.5 groups
                    for g in range(GPP):
                        pg = psum.tile([128, BN], f32, tag="pg", bufs=5,
                                       name=f"pg{p}{bn}{g}")
                        for k in range(KC):
                            leaf_mm(wp, bn, g, k, pg)
                        if g == 1:
                            # flush BEFORE tanh g1: the deferred combs read
                            # lh g4..g7 of the previous half, whose buffers
                            # later tanhs would recycle (lh pool bufs=5)
                            while pending:
                                pending.pop(0)()
                        if g == 3:
                            while pending3:
                                pending3.pop(0)()
                        lhs[g] = leaf_tanh(p, bn, g, pg)
                        if g in (4, 6):
                            il = (g - 4) // 2
                            emit_comb(p, bn, il, ith,
                                      lhs[2 * il], lhs[2 * il + 1])

                inth[(p, bn)] = ith

                if p < NPANEL - 1:
                    pending.append(make_deferred(p, bn, ith, lhs[4], lhs[5],
                                                 lhs[6], lhs[7]))
                elif bn < NBH - 1:
                    # bn0's root section rides inside bn1's leaf stream in
                    # two stages so its ACT-dependent matmuls (q3, prp)
                    # never stall the PE queue
                    def deferred_root_a(bn=bn, ith=ith, l4=lhs[4], l5=lhs[5],
                                        l6=lhs[6], l7=lhs[7]):
                        state = root_part_a(bn, ith, l4, l5, l6, l7)

                        def deferred_root_b(bn=bn, ith=ith, state=state):
                            root_part_b(bn, ith, *state)
                        pending3.append(deferred_root_b)
                    pending.append(deferred_root_a)
                else:
                    root_section(bn, ith, lhs[4], lhs[5], lhs[6], lhs[7])

    nc.compile()
    return nc


def _pack_weights(inp):
    import ml_dtypes

    f = np.float32
    bf = ml_dtypes.bfloat16
    leaf_b = np.asarray(inp["leaf_b"], f)
    int_W = np.asarray(inp["int_W"], f)
    int_b = np.asarray(inp["int_b"], f)
    root_W = np.asarray(inp["root_W"], f)
    root_b = np.asarray(inp["root_b"], f)
    leaf_Wp = np.asarray(inp["leaf_Wp"], f)
    leaf_bp = np.asarray(inp["leaf_bp"], f)
    int_Wp = np.asarray(inp["int_Wp"], f)
    int_bp = np.asarray(inp["int_bp"], f)
    root_Wp = np.asarray(inp["root_Wp"], f)
    root_bp = np.asarray(inp["root_bp"], f)

    w = {}
    # lwt[d, l*32+h] = leaf_W[l, d, h]; repack to
    # [p, panel*8192 + k*1024 + q] = lwt[k*128+p, panel*1024+q]
    lwt = np.asarray(inp["leaf_W"], f).transpose(1, 0, 2).reshape(D, L * H)
    w["lw"] = np.ascontiguousarray(
        lwt.reshape(KC, 128, NPANEL, 1024).transpose(1, 2, 0, 3).reshape(
            128, NPANEL * KC * 1024)
    ).astype(bf)

    cwall = np.zeros((128, CWALL), f)
    for i in range(I):
        for j in range(2):
            base = (2 * i + j) * 128
            # int_W chunk j of node i: rows (c*32+h) = child (4j+c) hidden h
            cwall[:, base:base + 32] = int_W[i, 128 * j:128 * (j + 1), :]
            for c in range(4):
                lv = 8 * i + 4 * j + c
                cwall[c * 32:(c + 1) * 32, base + 32 + 4 * j + c] = leaf_Wp[lv, :, 0]
    for q in range(NPANEL):
        base = CWN + q * 128
        cwall[:, base:base + 32] = root_W[128 * q:128 * (q + 1), :]
        for c in range(4):
            iv = 4 * q + c
            cwall[c * 32:(c + 1) * 32, base + 32 + 4 * q + c] = int_Wp[iv, :, 0]
    cwall[0:32, CWN + 512] = root_Wp[:, 0]
    w["cwall"] = cwall.astype(bf)

    biasall = np.zeros((128, 55), f)
    biasall[:, 0:32] = leaf_b.reshape(32, 128).T
    biasall[:, 32:36] = int_b.reshape(4, 128).T
    biasall[0:8, 36:52] = leaf_bp.reshape(16, 8).T
    biasall[0:16, 52] = int_bp[:, 0]
    biasall[0:32, 53] = root_b
    biasall[0, 54] = root_bp[0]
    w["biasall"] = biasall
    return w


def kernel(**inputs):
    import ml_dtypes

    from concourse.bass_utils import run_bass_kernel_spmd

    nc = _CACHE.get("nc")
    if nc is None:
        nc = _CACHE["nc"] = _build_nc()

    x = np.asarray(inputs["x"], np.float32)
    w = _pack_weights(inputs)
    in_maps = []
    for c in range(NCORES):
        m = dict(w)
        # [p, bn*4096 + k*512 + cc] = x[c*BC + bn*512 + cc, k*128 + p]
        xc = x[c * BC:(c + 1) * BC, :].reshape(NBH, BN, KC, 128)
        m["xt"] = np.ascontiguousarray(
            xc.transpose(3, 0, 2, 1).reshape(128, NBH * KC * BN)
        ).astype(ml_dtypes.bfloat16)
        in_maps.append(m)

    res = run_bass_kernel_spmd(nc, in_maps, core_ids=list(range(NCORES)))
    _CACHE["last_res"] = res
    outs = []
    for c in range(NCORES):
        # outl[r, (bn*16 + i)*512 + cc] -> leaf 8i+r at batch bn*512+cc
        ol = res.results[c]["outl"].reshape(8, NBH, I, BN)
        leafp = ol.transpose(2, 0, 1, 3).reshape(L, BC)
        outs.append(np.concatenate([leafp, res.results[c]["outi"]], axis=0))
    full = np.concatenate([o[:, :, None] for o in outs], axis=1)  # [145, B, 1]
    return full.astype(np.float32)


# revision 27
# speedup vs baseline: 1.1257x; 1.1257x over previous
"""Trainium2 Bass kernel for nn_CombineNode_7395933684091 (gnn_message_passing).

Hierarchy: 128 leaf terms (each D=1024 -> H=32), 16 internal terms
(concat of 8 children hiddens, 256 -> 32), 1 root (concat of 16
internal hiddens, 512 -> 32); every term also has a 1-dim predict head.
All matmuls followed by tanh.

Strategy: data-parallel over batch across 8 cores (Bc = 1024 rows per
core), weights replicated. On-chip layout keeps hidden features on the
PARTITION axis ("h^T layout": tiles are [features, batch]), so every
level's contraction is a natural PE matmul and the child-concat is just
stacking partition tiles.

All matmul operands are bf16 and every stationary operand is padded to
128 columns: FWL (fast weight load) then applies to every LDWEIGHTS,
and the PE never pays the ~95ns FWL<->non-FWL mode-switch penalty
(measured on the 64-col internal-level matmuls). PSUM stays fp32;
biases/activations/outputs stay fp32.

Inputs are host-packed into exact SBUF layouts so every load is a few
large contiguous DMAs split across both HWDGE queues (SP: weight
panels, ACT: x + combined consts) - each extra dma_start costs
~0.6-1.2us of queue issue time, so constants ride in two combined
tensors.

Leaf level: 4 panels x 8 groups (4 leaves) x 8 k-chunk accumulated
[128,128]x[128,512] matmuls. The per-term predict heads ride along as
extra block-diagonal columns fused into the internal-level stationary
operand (cw) and the root-level stationary operand (rw2), so they cost
no extra PE streaming. Internal-level (comb) matmuls trail their tanh
inputs by 2.5+ leaf groups (deferred emission), so the PE never waits
on the scalar engine; the last panel's root chain for batch-half 0 is
deferred into half 1's leaf stream for the same reason. Leaf
predictions land column-packed in an [8, 2*16*512] tile (engine
partition windows must be 32-aligned) and are unpacked on host.
"""

import numpy as np

B, D, H = 8192, 1024, 32
L, I, CPI = 128, 16, 8
NCORES = 8
BC = B // NCORES      # 1024 batch rows per core
BN = 512              # batch tile width (one PSUM bank of f32)
NBH = BC // BN        # 2 batch halves
KC = D // 128         # 8 contraction chunks for the leaf level
NPANEL = 4            # leaf panels (8 groups of 4 leaves each)
GPP = 8               # groups per panel
NOUT = L + I + 1      # 145
CWN = NPANEL * 2 * KC * 128   # 4096: cw region cols (32 chunks x 128)
CWALL = CWN + 512 + 128       # + rw2 (4x128) + rootwp block

_CACHE = {}


def _build_nc():
    from contextlib import ExitStack

    import concourse.mybir as mybir
    import concourse.tile as tile
    from concourse import bacc

    f32 = mybir.dt.float32
    bf16 = mybir.dt.bfloat16
    Tanh = mybir.ActivationFunctionType.Tanh

    nc = bacc.Bacc("TRN2", target_bir_lowering=False, debug=False)

    # xt: [128, bn*4096 + k*512 + c] = x[bn*512+c, k*128+p] (per-core slice)
    xt = nc.dram_tensor("xt", [128, NBH * KC * BN], bf16, kind="ExternalInput")
    # lw: [128, panel*8192 + k*1024 + g*128 + j]
    lw = nc.dram_tensor("lw", [128, NPANEL * KC * 1024], bf16, kind="ExternalInput")
    # cwall = cw (32 chunks x 128) | rw2 (4 x 128) | rootwp block (128)
    cwall = nc.dram_tensor("cwall", [128, CWALL], bf16, kind="ExternalInput")
    # biasall = lb (32) | intb (4) | lbp8 (16) | intbp | rootb | rootbp
    biasall = nc.dram_tensor("biasall", [128, 55], f32, kind="ExternalInput")
    # leaf predicts, column-packed: outl[r, (bn*16 + i)*512 + c] is leaf
    # 8i+r at batch bn*512+c
    outl = nc.dram_tensor("outl", [8, NBH * I * BN], f32, kind="ExternalOutput")
    # int predicts rows 0:16, root predict row 16
    outi = nc.dram_tensor("outi", [17, BC], f32, kind="ExternalOutput")

    mm = nc.tensor.matmul

    with tile.TileContext(nc) as tc, ExitStack() as ctx:
        consts = ctx.enter_context(tc.tile_pool(name="consts", bufs=1))
        wpool = ctx.enter_context(tc.tile_pool(name="wpool", bufs=3))
        work = ctx.enter_context(tc.tile_pool(name="work", bufs=5))
        keep = ctx.enter_context(tc.tile_pool(name="keep", bufs=1))
        psum = ctx.enter_context(tc.tile_pool(name="psum", bufs=1, space="PSUM"))

        # --- PE pre-warm: ~2.8us of dummy matmuls unthrottles the HAM clock
        # gate (PE boots at 1.2 GHz; 3.4us of sustained activity -> 2.4 GHz).
        # A gpsimd memset (done during the engine preamble) feeds the tile so
        # no DMA gates the first matmul.
        warm_c = consts.tile([128, 128], bf16, name="warm_c")
        nc.gpsimd.memset(warm_c[:], 0.0)
        pwarm = psum.tile([128, 128], f32, tag="pcomb", bufs=3, name="pwarm")
        for _ in range(26):
            mm(pwarm[:], warm_c[:], warm_c[:], start=True, stop=True,
               skip_group_check=True)

        # --- loads. ACT queue: x chunks + consts, ordered by first use;
        # SP queue: weight panels. bn0's x and panel 0's weights stream in
        # k-chunks so the first wave matmul only waits ~one chunk.
        # The first ~11us of HBM bandwidth belong to panel-0's weights and
        # bn0's x (the wave consumes them chunk-by-chunk); bn1's x follows
        # wp0 on the SP ring, and panels 1-2 ride the ACT ring behind the
        # consts so they can't steal bandwidth before ~20us.
        xt_sb = consts.tile([128, NBH * KC * BN], bf16, name="xt_sb")
        nc.scalar.dma_start(xt_sb[:, 0:1024], xt[:, 0:1024])
        nc.scalar.dma_start(xt_sb[:, 1024:2048], xt[:, 1024:2048])
        ball_sb = consts.tile([128, 55], f32, name="ball_sb")
        nc.scalar.dma_start(ball_sb[:], biasall[:])
        cwall_sb = consts.tile([128, CWALL], bf16, name="cwall_sb")
        nc.scalar.dma_start(cwall_sb[:, 0:1024], cwall[:, 0:1024])
        nc.scalar.dma_start(xt_sb[:, 2048:3072], xt[:, 2048:3072])
        nc.scalar.dma_start(xt_sb[:, 3072:4096], xt[:, 3072:4096])

        lb_sb = ball_sb[:, 0:32]
        intb_sb = ball_sb[:, 32:36]
        lbp8_sb = ball_sb[0:8, 36:52]
        intbp_sb = ball_sb[0:16, 52:53]
        rootb_sb = ball_sb[0:32, 53:54]
        rootbp_sb = ball_sb[0:1, 54:55]

        wps = {}
        wps[0] = wpool.tile([128, KC * 1024], bf16, tag="wpanel", name="wp0")
        nc.sync.dma_start(wps[0][:, 0:1024], lw[:, 0:1024])
        nc.sync.dma_start(wps[0][:, 1024:2048], lw[:, 1024:2048])
        nc.sync.dma_start(wps[0][:, 2048:4096], lw[:, 2048:4096])
        nc.sync.dma_start(wps[0][:, 4096:8192], lw[:, 4096:8192])
        nc.sync.dma_start(xt_sb[:, 4096:8192], xt[:, 4096:8192])

        wps[1] = wpool.tile([128, KC * 1024], bf16, tag="wpanel", name="wp1")
        nc.scalar.dma_start(wps[1][:], lw[:, 8192:16384])
        nc.scalar.dma_start(cwall_sb[:, 1024:CWALL], cwall[:, 1024:CWALL])
        wps[2] = wpool.tile([128, KC * 1024], bf16, tag="wpanel", name="wp2")
        nc.scalar.dma_start(wps[2][:], lw[:, 16384:24576])

        # leaf predictions, column-packed (bn-major) to keep every
        # activation write at partition base 0
        lpp = keep.tile([8, NBH * I * BN], f32, name="lpp")
        intp_sb = keep.tile([16, BC], f32, name="intp_sb")
        rootp_sb = keep.tile([1, BC], f32, name="rootp_sb")

        inth = {}  # (panel, bn) -> [128, BN] tile: int nodes 4p..4p+3 h^T

        def leaf_mm(wp, bn, gl, k, pg):
            mm(
                pg[:],
                wp[:, k * 1024 + gl * 128:k * 1024 + (gl + 1) * 128],
                xt_sb[:, bn * 4096 + k * BN:bn * 4096 + (k + 1) * BN],
                start=(k == 0),
                stop=(k == KC - 1),
            )

        def leaf_tanh(p, bn, gl, pg):
            lh = work.tile([128, BN], bf16, tag="lh", name=f"lh{p}{bn}{gl}")
            nc.scalar.activation(
                lh[:], pg[:], Tanh, bias=lb_sb[:, GPP * p + gl:GPP * p + gl + 1]
            )
            return lh

        def comb_mms(p, bn, il, lh0, lh1):
            """Fused internal-trans + leaf-predict matmul pair.

            pcomb rows 0:32 accumulate node (4p+il)'s hidden pre-activation
            over its two child groups; rows 32:40 pick up the 8 leaf predict
            dots via the block-diagonal columns (rest of the 128-col padded
            stationary is zero)."""
            i = 4 * p + il
            pcomb = psum.tile([128, BN], f32, tag="pcomb", bufs=3,
                              name=f"pc{p}{bn}{il}")
            for j, lh in ((0, lh0), (1, lh1)):
                mm(
                    pcomb[:],
                    cwall_sb[:, (2 * i + j) * 128:(2 * i + j + 1) * 128],
                    lh[:],
                    start=(j == 0),
                    stop=(j == 1),
                    skip_group_check=True,
                )
            return pcomb

        def comb_ith(p, bn, il, ith, pcomb):
            nc.scalar.activation(
                ith[32 * il:32 * il + 32, :],
                pcomb[0:32, :],
                Tanh,
                bias=intb_sb[32 * il:32 * il + 32, p:p + 1],
            )

        def comb_lpp(p, bn, il, pcomb):
            i = 4 * p + il
            nc.scalar.activation(
                lpp[0:8, (bn * I + i) * BN:(bn * I + i + 1) * BN],
                pcomb[32:40, :],
                Tanh,
                bias=lbp8_sb[:, i:i + 1],
            )

        def emit_comb(p, bn, il, ith, lh0, lh1):
            pcomb = comb_mms(p, bn, il, lh0, lh1)
            comb_ith(p, bn, il, ith, pcomb)
            comb_lpp(p, bn, il, pcomb)

        def flush_outl(p, bn, eng):
            lo = (bn * I + 4 * p) * BN
            hi = (bn * I + 4 * (p + 1)) * BN
            eng.dma_start(outl[0:8, lo:hi], lpp[0:8, lo:hi])

        def make_deferred(p, bn, ith, lh4, lh5, lh6, lh7):
            # il2 + il3 combs + the predict flush for (p, bn), emitted while
            # the NEXT half's leaf stream occupies the PE (their tanh inputs
            # are then guaranteed ready)
            def deferred():
                emit_comb(p, bn, 2, ith, lh4, lh5)
                emit_comb(p, bn, 3, ith, lh6, lh7)
                flush_outl(p, bn, nc.gpsimd)
            return deferred

        def root_part_a(bn, ith, lh4, lh5, lh6, lh7):
            """Last panel's il2/il3 combs + root accumulation q=0..2 + the
            ith activations. Root q0..q2 bridge the tanh-g7 wait."""
            p = NPANEL - 1
            pc2 = comb_mms(p, bn, 2, lh4, lh5)
            prc = psum.tile([128, BN], f32, tag="pg", bufs=5, name=f"prc{bn}")
            for q in range(NPANEL - 1):
                mm(prc[:], cwall_sb[:, CWN + 128 * q:CWN + 128 * (q + 1)],
                   inth[(q, bn)][:], start=(q == 0), stop=False,
                   skip_group_check=True)
            pc3 = comb_mms(p, bn, 3, lh6, lh7)
            comb_ith(p, bn, 2, ith, pc2)
            comb_ith(p, bn, 3, ith, pc3)
            return prc, pc2, pc3

        def root_part_b(bn, ith, prc, pc2, pc3):
            """Root q3 + the predict/root activation chain + output DMAs.
            Flushed two groups after part A so q3/prp never wait on ACT."""
            eng = nc.scalar if bn == NBH - 1 else nc.gpsimd
            p = NPANEL - 1
            mm(prc[:], cwall_sb[:, CWN + 384:CWN + 512], ith[:], start=False,
               stop=True, skip_group_check=True)
            rh = work.tile([32, BN], bf16, tag="rh", name=f"rh{bn}")
            nc.scalar.activation(rh[:], prc[0:32, :], Tanh,
                                 bias=rootb_sb[:, 0:1])
            prp = psum.tile([128, BN], f32, tag="pg", bufs=5, name=f"prp{bn}")
            mm(prp[:], cwall_sb[0:32, CWN + 512:CWN + 640], rh[:],
               start=True, stop=True, skip_group_check=True)
            comb_lpp(p, bn, 2, pc2)
            comb_lpp(p, bn, 3, pc3)
            flush_outl(p, bn, eng)
            nc.scalar.activation(
                intp_sb[:, bn * BN:bn * BN + BN], prc[32:48, :],
                Tanh, bias=intbp_sb[:, 0:1],
            )
            eng.dma_start(
                outi[0:16, bn * BN:bn * BN + BN],
                intp_sb[:, bn * BN:bn * BN + BN],
            )
            nc.scalar.activation(
                rootp_sb[0:1, bn * BN:bn * BN + BN], prp[0:1, :], Tanh,
                bias=rootbp_sb[:, 0:1],
            )
            eng.dma_start(
                outi[16:17, bn * BN:bn * BN + BN],
                rootp_sb[0:1, bn * BN:bn * BN + BN],
            )

        def root_section(bn, ith, lh4, lh5, lh6, lh7):
            prc, pc2, pc3 = root_part_a(bn, ith, lh4, lh5, lh6, lh7)
            root_part_b(bn, ith, prc, pc2, pc3)

        pending = []   # deferred combs/flush from the previous (p, bn), at g1
        pending3 = []  # deferred root part B (bn0), flushed at g3

        # --- leaf + internal levels ----------------------------------------
        for p in range(NPANEL):
            if p in wps:
                wp = wps[p]
            else:
                wp = wpool.tile([128, KC * 1024], bf16, tag="wpanel", name=f"wp{p}")
                nc.sync.dma_start(wp[:], lw[:, p * 8192:(p + 1) * 8192])
            for bn in range(NBH):
                ith = keep.tile([128, BN], bf16, tag=f"inth{p}{bn}",
                                name=f"inth{p}{bn}")
                lhs = {}

                if p == 0 and bn == 0:
                    # k-outer waves (5 then 3 groups): matmuls start as soon
                    # as the first xt/wp chunks land, and each arriving chunk
                    # feeds a full wave of matmuls. il0/il1 combs run between
                    # the waves; il2/il3 are deferred like everywhere else.
                    pgs = {}
                    for g0, cnt in ((0, 5), (5, 3)):
                        for q in range(cnt):
                            pgs[g0 + q] = psum.tile(
                                [128, BN], f32, tag="pg", bufs=5,
                                name=f"pgko{g0 + q}")
                        for k in range(KC):
                            for q in range(cnt):
                                leaf_mm(wp, bn, g0 + q, k, pgs[g0 + q])
                        if g0 == 0:
                            for g in (0, 1, 2, 3):
                                lhs[g] = leaf_tanh(p, bn, g, pgs[g])
                            for il in (0, 1):
                                emit_comb(p, bn, il, ith,
                                          lhs[2 * il], lhs[2 * il + 1])
                            lhs[4] = leaf_tanh(p, bn, 4, pgs[4])
                        else:
                            for g in (5, 6, 7):
                                lhs[g] = leaf_tanh(p, bn, g, pgs[g])
                else:
                    # in-loop combs trail their tanh inputs by 2.5 groups
                    for g in range(GPP):
                        pg = psum.tile([128, BN], f32, tag="pg", bufs=5,
                                       name=f"pg{p}{bn}{g}")
                        for k in range(KC):
                            leaf_mm(wp, bn, g, k, pg)
                        if g == 1:
                            # flush BEFORE tanh g1: the deferred combs read
                            # lh g4..g7 of the previous half, whose buffers
                            # later tanhs would recycle (lh pool bufs=5)
                            while pending:
                                pending.pop(0)()
                        if g == 3:
                            while pending3:
                                pending3.pop(0)()
                        lhs[g] = leaf_tanh(p, bn, g, pg)
                        if g in (4, 6):
                            il = (g - 4) // 2
                            emit_comb(p, bn, il, ith,
                                      lhs[2 * il], lhs[2 * il + 1])

                inth[(p, bn)] = ith

                if p < NPANEL - 1:
                    pending.append(make_deferred(p, bn, ith, lhs[4], lhs[5],
                                                 lhs[6], lhs[7]))
                elif bn < NBH - 1:
                    # bn0's root section rides inside bn1's leaf stream in
                    # two stages so its ACT-dependent matmuls (q3, prp)
                    # never stall the PE queue
                    def deferred_root_a(bn=bn, ith=ith, l4=lhs[4], l5=lhs[5],
                                        l6=lhs[6], l7=lhs[7]):
                        state = root_part_a(bn, ith, l4, l5, l6, l7)

                        def deferred_root_b(bn=bn, ith=ith, state=state):
                            root_part_b(bn, ith, *state)
                        pending3.append(deferred_root_b)
                    pending.append(deferred_root_a)
                else:
                    root_section(bn, ith, lhs[4], lhs[5], lhs[6], lhs[7])

    nc.compile()
    return nc


def _pack_weights(inp):
    import ml_dtypes

    f = np.float32
    bf = ml_dtypes.bfloat16
    leaf_b = np.asarray(inp["leaf_b"], f)
    int_W = np.asarray(inp["int_W"], f)
    int_b = np.asarray(inp["int_b"], f)
    root_W = np.asarray(inp["root_W"], f)
    root_b = np.asarray(inp["root_b"], f)
    leaf_Wp = np.asarray(inp["leaf_Wp"], f)
    leaf_bp = np.asarray(inp["leaf_bp"], f)
    int_Wp = np.asarray(inp["int_Wp"], f)
    int_bp = np.asarray(inp["int_bp"], f)
    root_Wp = np.asarray(inp["root_Wp"], f)
    root_bp = np.asarray(inp["root_bp"], f)

    w = {}
    # lwt[d, l*32+h] = leaf_W[l, d, h]; repack to
    # [p, panel*8192 + k*1024 + q] = lwt[k*128+p, panel*1024+q]
    lwt = np.asarray(inp["leaf_W"], f).transpose(1, 0, 2).reshape(D, L * H)
    w["lw"] = np.ascontiguousarray(
        lwt.reshape(KC, 128, NPANEL, 1024).transpose(1, 2, 0, 3).reshape(
            128, NPANEL * KC * 1024)
    ).astype(bf)

    cwall = np.zeros((128, CWALL), f)
    for i in range(I):
        for j in range(2):
            base = (2 * i + j) * 128
            # int_W chunk j of node i: rows (c*32+h) = child (4j+c) hidden h
            cwall[:, base:base + 32] = int_W[i, 128 * j:128 * (j + 1), :]
            for c in range(4):
                lv = 8 * i + 4 * j + c
                cwall[c * 32:(c + 1) * 32, base + 32 + 4 * j + c] = leaf_Wp[lv, :, 0]
    for q in range(NPANEL):
        base = CWN + q * 128
        cwall[:, base:base + 32] = root_W[128 * q:128 * (q + 1), :]
        for c in range(4):
            iv = 4 * q + c
            cwall[c * 32:(c + 1) * 32, base + 32 + 4 * q + c] = int_Wp[iv, :, 0]
    cwall[0:32, CWN + 512] = root_Wp[:, 0]
    w["cwall"] = cwall.astype(bf)

    biasall = np.zeros((128, 55), f)
    biasall[:, 0:32] = leaf_b.reshape(32, 128).T
    biasall[:, 32:36] = int_b.reshape(4, 128).T
    biasall[0:8, 36:52] = leaf_bp.reshape(16, 8).T
    biasall[0:16, 52] = int_bp[:, 0]
    biasall[0:32, 53] = root_b
    biasall[0, 54] = root_bp[0]
    w["biasall"] = biasall
    return w


def kernel(**inputs):
    import ml_dtypes

    from concourse.bass_utils import run_bass_kernel_spmd

    nc = _CACHE.get("nc")
    if nc is None:
        nc = _CACHE["nc"] = _build_nc()

    x = np.asarray(inputs["x"], np.float32)
    w = _pack_weights(inputs)
    in_maps = []
    for c in range(NCORES):
        m = dict(w)
        # [p, bn*4096 + k*512 + cc] = x[c*BC + bn*512 + cc, k*128 + p]
        xc = x[c * BC:(c + 1) * BC, :].reshape(NBH, BN, KC, 128)
        m["xt"] = np.ascontiguousarray(
            xc.transpose(3, 0, 2, 1).reshape(128, NBH * KC * BN)
        ).astype(ml_dtypes.bfloat16)
        in_maps.append(m)

    res = run_bass_kernel_spmd(nc, in_maps, core_ids=list(range(NCORES)))
    _CACHE["last_res"] = res
    outs = []
    for c in range(NCORES):
        # outl[r, (bn*16 + i)*512 + cc] -> leaf 8i+r at batch bn*512+cc
        ol = res.results[c]["outl"].reshape(8, NBH, I, BN)
        leafp = ol.transpose(2, 0, 1, 3).reshape(L, BC)
        outs.append(np.concatenate([leafp, res.results[c]["outi"]], axis=0))
    full = np.concatenate([o[:, :, None] for o in outs], axis=1)  # [145, B, 1]
    return full.astype(np.float32)
